# revision 15
# baseline (speedup 1.0000x reference)
"""Trainium2 Bass kernel: ViT-style multimodal transformer (12L, D=768, H=12).

Strategy: pure data parallel - 8 batch elements, one per NeuronCore.
Each core runs the full transformer on its [667, 768] token sequence.

v2 over the previous baseline (2.98ms):
  - All lin_T weight column-blocks are pre-packed on the host so every
    weight DMA is a contiguous [128, 768] row-block on HWDGE (the old
    SWDGE strided loads emitted 768 x 256B descriptors per tile and kept
    the Q7 descriptor engine ~38% busy).
  - LN2 is fused into the proj lin_N and LN1 of layer l+1 into FFN2 of
    layer l (post_tile): the PE transposes interleave with matmul groups
    instead of forming 9us transpose-only windows, which kept re-arming
    the HAM clock throttle (PE was at 1.2GHz 57% of the time).
  - Per-tile LN transposes write one merged [128, 6, 128] bf16 psum tile,
    evacuated with a single ACT copy (6x fewer ACT instructions; ACT has
    a 352-cycle fixed cost per instruction).
  - lin_T evacuations (K/Q copy, FFN1 relu+bias) moved from ACT to DVE.
  - K and Q projections are emitted inside the attention pair pipeline
    (pair j+2's K/Q between pair j's AV halves) so the tensor engine has
    ~8.4us of matmul work per pair to cover the ~10us of exp on ACT.
  - The ones-block in V now occupies columns 0:64, so the softmax
    denominator lands on partitions 0:63 and the DVE reciprocal runs
    directly on the PSUM accumulator (the old ACT partition-bounce is
    gone).

Token order is permuted (attention is permutation-equivariant; positional
embeddings are baked into the additive base): [obs(392) | goal(196) | cls |
pose | text(77)], so patch embeddings land partition-aligned. cls lives at
row 588 = (j=4, p=76).
"""

import numpy as np
import ml_dtypes

import concourse.bass as bass
import concourse.bacc as bacc_mod
import concourse.mybir as mybir
import concourse.tile as tile
from concourse.bass_utils import run_bass_kernel_spmd
from concourse.masks import make_identity

BF16 = mybir.dt.bfloat16
F32 = mybir.dt.float32
AF = mybir.ActivationFunctionType
ALU = mybir.AluOpType

L, H, D, HD = 12, 12, 768, 64
P, IMG, NP, HS = 16, 224, 196, 2
TBLK, VOCAB, POSE_DIM, OUT = 77, 96, 7, 7
B = 8
SEQ = 667          # 1 cls + 1 pose + 392 obs + 77 text + 196 goal
TPAD = 768         # padded token slots (6 partition tiles)
SPAD = 672         # padded free-dim length of transposed activations
NT = 6             # token partition tiles
ND = 6             # feature partition tiles (768/128)
NF = 24            # ffn feature tiles (3072/128)
SCALE = float(D) ** -0.5
EPS = 1e-5
CLS = 588          # permuted cls position = (tile 4, row 76)
CLS_J, CLS_P = 4, 76

# token tiles (start, width)
TT = [(0, 128), (128, 128), (256, 128), (384, 128), (512, 128), (640, 27)]
CH_T = [(0, 512), (512, 155)]   # SEQ chunks (psum bank = 512 fp32)
CH_D = [(0, 512), (512, 256)]   # D chunks
CH_CLS = [(CLS, 1)]             # cls-only chunk (last layer)
# 3-way SEQ split: chunk A only needs token tiles 0-2, B tile 3, C tiles 4-5.
# Offsets keep every psum write inside one 2KB bank.
CH_T3 = [(0, 384), (384, 128), (512, 155)]

# Runtime knobs (test.py may flip these)
TRACE = False
TRACE_CORES = [0]
CLS_LAST = True
LAST_EXEC_NS = None
LAST_TRACE_PATH = None
_CACHE = {}


def _bcast128(ap1d):
    """DMA access pattern broadcasting a 1-D DRAM row across 128 partitions."""
    return bass.AP(tensor=ap1d.tensor, offset=ap1d.offset,
                   ap=[[0, 128]] + list(ap1d.ap))


def build_nc(has_gb=False, has_bias=False, layers=L, cls_last=True):
    nc = bacc_mod.Bacc()

    # ---- per-core data inputs ----
    base = nc.declare_dram_parameter("base", [TPAD, D], F32, isOutput=False)
    pobsT = nc.declare_dram_parameter("pobsT", [D, 392], BF16, isOutput=False)
    pgoalT = nc.declare_dram_parameter("pgoalT", [D, 204], BF16, isOutput=False)
    # ---- shared weights ----
    obs_w = nc.declare_dram_parameter("obs_w", [D, D], BF16, isOutput=False)
    goal_w = nc.declare_dram_parameter("goal_w", [D, D], BF16, isOutput=False)
    # packed column-block layouts: w_p[l, n, kp, kt*128 + c] = w[l, kt*128+kp, n*128+c]
    wqp = nc.declare_dram_parameter("wqp", [L, ND, 128, D], BF16, isOutput=False)
    wkp = nc.declare_dram_parameter("wkp", [L, ND, 128, D], BF16, isOutput=False)
    fw1p = nc.declare_dram_parameter("fw1p", [L, NF, 128, D], BF16, isOutput=False)
    # k-row layouts (lin_N streams row blocks)
    wv = nc.declare_dram_parameter("wv", [L, D, D], BF16, isOutput=False)
    pw = nc.declare_dram_parameter("pw", [L, D, D], BF16, isOutput=False)
    fw2 = nc.declare_dram_parameter("fw2", [L, 4 * D, D], BF16, isOutput=False)
    if has_bias:
        pb = nc.declare_dram_parameter("pb", [L, D], F32, isOutput=False)
        fb1 = nc.declare_dram_parameter("fb1", [L, 4 * D], F32, isOutput=False)
        fb2 = nc.declare_dram_parameter("fb2", [L, D], F32, isOutput=False)
    if has_gb:
        ln1g = nc.declare_dram_parameter("ln1g", [L, D], F32, isOutput=False)
        ln1b = nc.declare_dram_parameter("ln1b", [L, D], F32, isOutput=False)
        ln2g = nc.declare_dram_parameter("ln2g", [L, D], F32, isOutput=False)
        ln2b = nc.declare_dram_parameter("ln2b", [L, D], F32, isOutput=False)
    clsout = nc.declare_dram_parameter("clsout", [1, D], F32, isOutput=True)

    with tile.TileContext(nc) as tc:
        with (
            tc.tile_pool(name="singles", bufs=1) as singles,
            tc.tile_pool(name="wblk", bufs=4) as wblk,    # lin_T weight tiles
            tc.tile_pool(name="rhsk", bufs=12) as rhsk,   # lin_N weight k-tiles
            tc.tile_pool(name="upool", bufs=4) as upool,  # exp(S^T) per head
            tc.tile_pool(name="hn", bufs=6) as hn,        # deferred LN tiles
            tc.tile_pool(name="rows", bufs=2) as rows,
            tc.tile_pool(name="stats", bufs=6) as stats,
            tc.tile_pool(name="lnv", bufs=4) as lnv,
            tc.tile_pool(name="pp", bufs=4, space="PSUM") as pp,
        ):
            # ---------- persistent SBUF ----------
            ident = singles.tile([128, 128], BF16)
            make_identity(nc, ident)
            eps_sb = singles.tile([128, 1], F32)
            nc.vector.memset(eps_sb, EPS)

            x = singles.tile([128, NT, D], F32)            # residual stream
            hT = singles.tile([128, ND, SPAD], BF16)       # LN output, transposed
            QT = singles.tile([128, ND, SPAD], BF16)
            KT = singles.tile([128, ND, SPAD], BF16)
            vbuf = singles.tile([128, NT, H, 2 * HD], BF16)  # ones block + V
            OT = singles.tile([128, ND, SPAD], BF16)       # attn out, transposed
            h3T = singles.tile([128, NF, SPAD], BF16)      # relu ffn hidden, transposed

            # ones block FIRST (cols 0:HD): the AV matmul broadcasts the
            # softmax denominator into PSUM partitions 0..63 so the DVE
            # reciprocal runs there directly (no partition bounce).
            for t_i in range(NT):
                nc.vector.memset(vbuf[:, t_i, :, 0:HD], 1.0)

            # ---------- load residual base ----------
            nc.sync.dma_start(out=x[:], in_=base.rearrange("(j p) d -> p j d", p=128))

            # ---------- patch embeddings ----------
            pobs_sb = singles.tile([128, ND, 392], BF16)
            nc.sync.dma_start(out=pobs_sb[:],
                              in_=pobsT.rearrange("(kt kp) t -> kp kt t", kp=128))
            pgoal_sb = singles.tile([128, ND, 204], BF16)
            nc.sync.dma_start(out=pgoal_sb[:],
                              in_=pgoalT.rearrange("(kt kp) t -> kp kt t", kp=128))

            def embed_add(psrc_sb, w_dram, ptiles, dests):
                # ptiles: list of (col0, width); dests: list of (xrow0, xj)
                for gi in range(0, len(ptiles), 2):
                    grp = list(range(gi, min(gi + 2, len(ptiles))))
                    psums = {}
                    for t_i in grp:
                        psums[t_i] = pp.tile([128, D], F32, tag="b",
                                             name=f"pe{t_i}")
                    for k in range(ND):
                        wk_t = rhsk.tile([128, D], BF16, tag="rhsk")
                        nc.sync.dma_start(out=wk_t[:],
                                          in_=w_dram[k * 128:(k + 1) * 128, :])
                        for t_i in grp:
                            c0, cw = ptiles[t_i]
                            for (s, w) in CH_D:
                                nc.tensor.matmul(
                                    psums[t_i][:cw, s:s + w],
                                    lhsT=psrc_sb[:, k, c0:c0 + cw],
                                    rhs=wk_t[:, s:s + w],
                                    start=(k == 0), stop=(k == ND - 1))
                    for t_i in grp:
                        c0, cw = ptiles[t_i]
                        r0, xj = dests[t_i]
                        nc.vector.tensor_add(
                            out=x[r0:r0 + cw, xj, :],
                            in0=x[r0:r0 + cw, xj, :],
                            in1=psums[t_i][:cw, :])

            embed_add(pobs_sb, obs_w,
                      [(0, 128), (128, 128), (256, 128), (384, 8)],
                      [(0, 0), (0, 1), (0, 2), (0, 3)])
            embed_add(pgoal_sb, goal_w,
                      [(0, 128), (128, 76)],
                      [(0, 3), (0, 4)])

            # ---------- helpers ----------
            def ln_chain_tile(ti, t0, tw, g_bc, b_bc):
                """LN stats + normalize for one tile (DVE/ACT/GPSIMD only).

                Returns a finisher emitting the PE transposes + hT copy;
                the caller defers it under later matmul work so the tensor
                engine never waits on this chain.
                """
                st = stats.tile([128, 3, 6], F32, tag="bnst")
                mv = stats.tile([128, 2], F32, tag="bnmv")
                rstd = stats.tile([128, 1], F32, tag="rstd")
                xi = x[:tw, ti, :].rearrange("p (s c) -> p s c", s=3)
                for s in range(3):
                    nc.vector.bn_stats(out=st[:tw, s, :], in_=xi[:, s, :])
                nc.vector.bn_aggr(out=mv[:tw], in_=st[:tw])
                nc.scalar.activation(out=rstd[:tw], in_=mv[:tw, 1:2],
                                     func=AF.Sqrt, bias=eps_sb[:tw], scale=1.0)
                nc.vector.reciprocal(out=rstd[:tw], in_=rstd[:tw])
                hnat = hn.tile([128, D], BF16, tag="hnat")
                # normalize on GPSIMD (SBUF->SBUF): keeps DVE free for evacs
                nc.gpsimd.tensor_scalar(out=hnat[:tw], in0=x[:tw, ti, :],
                                        scalar1=mv[:tw, 0:1], scalar2=rstd[:tw],
                                        op0=ALU.subtract, op1=ALU.mult)
                if has_gb:
                    nc.gpsimd.tensor_mul(out=hnat[:tw], in0=hnat[:tw],
                                         in1=g_bc[:tw])
                    nc.gpsimd.tensor_add(out=hnat[:tw], in0=hnat[:tw],
                                         in1=b_bc[:tw])

                def fin():
                    # merged [128, ND, 128] bf16 psum tile, one ACT copy
                    ptile = pp.tile([128, ND, 128], BF16, tag="b", name="pt")
                    for dj in range(ND):
                        nc.tensor.transpose(ptile[:, dj, :tw],
                                            hnat[:tw, dj * 128:(dj + 1) * 128],
                                            ident[:tw, :tw])
                    nc.scalar.activation(out=hT[:, :, t0:t0 + tw],
                                         in_=ptile[:, :, :tw], func=AF.Copy)
                return fin

            def ln_loader(g_dram, b_dram):
                g_bc = b_bc = None
                if has_gb:
                    g_bc = lnv.tile([128, D], F32, tag="g")
                    b_bc = lnv.tile([128, D], F32, tag="bb")
                    nc.sync.dma_start(out=g_bc[:], in_=_bcast128(g_dram))
                    nc.sync.dma_start(out=b_bc[:], in_=_bcast128(b_dram))
                return g_bc, b_bc

            def layer_norm_into_hT(g_dram=None, b_dram=None, tiles=None):
                """Standalone LN (initial layer + cls tile): immediate fin."""
                g_bc, b_bc = ln_loader(g_dram, b_dram)
                for ti, (t0, tw) in enumerate(TT):
                    if tiles is not None and ti not in tiles:
                        continue
                    ln_chain_tile(ti, t0, tw, g_bc, b_bc)()

            def make_ln_chain(g_dram, b_dram, bias_dram=None):
                """chain callback for lin_N: optional residual bias + LN."""
                g_bc, b_bc = ln_loader(g_dram, b_dram)
                bias_bc = None
                if bias_dram is not None:
                    bias_bc = lnv.tile([128, D], F32, tag="xb")
                    nc.sync.dma_start(out=bias_bc[:], in_=_bcast128(bias_dram))

                def chain(t_i, t0, tw):
                    if bias_bc is not None:
                        nc.vector.tensor_add(out=x[:tw, t_i, :],
                                             in0=x[:tw, t_i, :],
                                             in1=bias_bc[:tw])
                    return ln_chain_tile(t_i, t0, tw, g_bc, b_bc)
                return chain

            def lin_T_n(wp_ln, out_sb, n, src=None, relu=False, bias_col=None,
                        ch=CH_T, chunk_pending=None):
                """One n-tile of a transposed-output linear: out[:, n, t].

                wp_ln: packed DRAM slice [128, k_tiles*128] (contiguous).
                chunk_pending: list of finisher closures emitted after the
                first chunk's k-loop (chunk-major mode) - used to slot
                deferred LN transposes under this tile's matmuls.
                """
                if src is None:
                    src = hT
                k_tiles = src.shape[1]
                wb = wblk.tile([128, k_tiles * 128], BF16, tag="wblk")
                nc.sync.dma_start(out=wb[:], in_=wp_ln)
                ps = pp.tile([128, D], F32, tag="b", name="lt")
                if chunk_pending is not None:
                    for ci, (c0, w) in enumerate(ch):
                        for k in range(k_tiles):
                            nc.tensor.matmul(
                                ps[:, c0:c0 + w],
                                lhsT=wb[:, k * 128:(k + 1) * 128],
                                rhs=src[:, k, c0:c0 + w],
                                start=(k == 0), stop=(k == k_tiles - 1))
                        if ci == 0:
                            for fin in chunk_pending:
                                fin()
                else:
                    for k in range(k_tiles):
                        for (c0, w) in ch:
                            nc.tensor.matmul(
                                ps[:, c0:c0 + w],
                                lhsT=wb[:, k * 128:(k + 1) * 128],
                                rhs=src[:, k, c0:c0 + w],
                                start=(k == 0), stop=(k == k_tiles - 1))
                lo, hi = ch[0][0], ch[-1][0] + ch[-1][1]
                if relu:
                    if bias_col is not None:
                        nc.vector.tensor_scalar(
                            out=out_sb[:, n, lo:hi], in0=ps[:, lo:hi],
                            scalar1=bias_col[:, n:n + 1], scalar2=0.0,
                            op0=ALU.add, op1=ALU.max)
                    else:
                        nc.vector.tensor_scalar_max(
                            out=out_sb[:, n, lo:hi], in0=ps[:, lo:hi],
                            scalar1=0.0)
                else:
                    nc.vector.tensor_copy(out=out_sb[:, n, lo:hi],
                                          in_=ps[:, lo:hi])

            def lin_N(w_dram_l, src_sb, k_tiles, evac, tiles=None, chain=None):
                """Natural-layout output: psum[t, 0:768] = src.T @ w per token tile.

                chain(t_i, t0, tw) emits the fused-LN stats chain (DVE/ACT)
                right after each tile's evacuation and returns a finisher
                (PE transposes). Finishers of group g are emitted after
                group g+1's matmuls so the PE never waits on the chain; the
                last group's finishers are RETURNED for the caller to emit
                under the next phase's matmuls.
                """
                tlist = [(t_i, t0, tw) for t_i, (t0, tw) in enumerate(TT)
                         if tiles is None or t_i in tiles]
                prev_fins = []
                for gi in range(0, len(tlist), 3):
                    grp = tlist[gi:gi + 3]
                    psums = {}
                    for (t_i, t0, tw) in grp:
                        psums[t_i] = pp.tile([128, D], F32, tag="b",
                                             name=f"ln{t_i}")
                    for k in range(k_tiles):
                        wk_t = rhsk.tile([128, D], BF16, tag="rhsk")
                        nc.sync.dma_start(out=wk_t[:],
                                          in_=w_dram_l[k * 128:(k + 1) * 128, :])
                        for (t_i, t0, tw) in grp:
                            for (c0, w) in CH_D:
                                nc.tensor.matmul(
                                    psums[t_i][:tw, c0:c0 + w],
                                    lhsT=src_sb[:, k, t0:t0 + tw],
                                    rhs=wk_t[:, c0:c0 + w],
                                    start=(k == 0), stop=(k == k_tiles - 1))
                    fins = []
                    for (t_i, t0, tw) in grp:
                        evac(t_i, t0, tw, psums[t_i])
                        if chain is not None:
                            fins.append(chain(t_i, t0, tw))
                    for fin in prev_fins:
                        fin()
                    prev_fins = fins
                return prev_fins

            def evac_accum_x(t_i, t0, tw, ps):
                nc.vector.tensor_add(out=x[:tw, t_i, :], in0=x[:tw, t_i, :],
                                     in1=ps[:tw, :])

            def evac_v(t_i, t0, tw, ps):
                nc.vector.tensor_copy(
                    out=vbuf[:tw, t_i, :, HD:2 * HD],
                    in_=ps[:tw, :].rearrange("p (h d) -> p h d", d=HD))

            def add_bias_to_x(b_dram_l, tiles=None):
                b_bc = lnv.tile([128, D], F32, tag="xb")
                nc.sync.dma_start(out=b_bc[:], in_=_bcast128(b_dram_l))
                for t_i, (t0, tw) in enumerate(TT):
                    if tiles is not None and t_i not in tiles:
                        continue
                    nc.vector.tensor_add(out=x[:tw, t_i, :], in0=x[:tw, t_i, :],
                                         in1=b_bc[:tw])

            def attention_kq(l, ch, s_tiles=TT, post_st0=None):
                """Attention with K/Q projections interleaved into the pair
                pipeline (writes OT). `ch` is the QUERY chunk list (cls-only
                on the last layer); K is always computed over all tokens.

                Pair j's S^T matmuls run one pair ahead of AV; K/Q of pair
                j+2 are emitted between AV halves so the tensor engine has
                matmul work covering exp's ACT latency. post_st0 (the V
                second half) is emitted right after S^T(0) so pair 0's exp
                runs under ~9us of matmuls. The interleave order also keeps
                the psum pool rotation deadlock-free: every slot's consumer
                is emitted within a few allocations.
                """
                lo, hi = ch[0][0], ch[-1][0] + ch[-1][1]

                def emit_kq(j):
                    lin_T_n(wkp[l, j], KT, j, ch=CH_T)
                    lin_T_n(wqp[l, j], QT, j, ch=ch)

                def emit_st_si(j, s_i, u_a, u_b):
                    s0, sw = s_tiles[s_i]
                    pa = pp.tile([128, D], F32, tag="b", name="pa")
                    pb_ = pp.tile([128, D], F32, tag="b", name="pb")
                    for (c0, w) in ch:
                        nc.tensor.matmul(pa[:sw, c0:c0 + w],
                                         lhsT=KT[0:64, j, s0:s0 + sw],
                                         rhs=QT[0:64, j, c0:c0 + w],
                                         start=True, stop=True)
                        nc.tensor.matmul(pb_[:sw, c0:c0 + w],
                                         lhsT=KT[64:128, j, s0:s0 + sw],
                                         rhs=QT[64:128, j, c0:c0 + w],
                                         start=True, stop=True)
                    nc.scalar.activation(out=u_a[:sw, s_i, lo:hi],
                                         in_=pa[:sw, lo:hi],
                                         func=AF.Exp, scale=SCALE)
                    nc.scalar.activation(out=u_b[:sw, s_i, lo:hi],
                                         in_=pb_[:sw, lo:hi],
                                         func=AF.Exp, scale=SCALE)

                def emit_avmm_half(j, h, u_h, po, half):
                    """Half of the AV matmuls for one head; partitions 0..63
                    get the denominator (ones block of vbuf)."""
                    ns_ = len(s_tiles)
                    rng = range(0, (ns_ + 1) // 2) if half == 0 else \
                        range((ns_ + 1) // 2, ns_)
                    for s_i in rng:
                        s0, sw = s_tiles[s_i]
                        for (c0, w) in ch:
                            nc.tensor.matmul(po[:, c0:c0 + w],
                                             lhsT=vbuf[:sw, s_i, h, :],
                                             rhs=u_h[:sw, s_i, c0:c0 + w],
                                             start=(s_i == 0),
                                             stop=(s_i == ns_ - 1))

                def emit_norm(j, h, po):
                    """OT rows = po[64:128] * 1/po[0:64] (denominator block)."""
                    r = (h % 2) * 64
                    rbc = rows.tile([128, SPAD], F32, tag="rb")
                    nc.vector.reciprocal_approx_fast(
                        out=rbc[0:HD, lo:hi], in_=po[0:HD, lo:hi])
                    nc.vector.tensor_mul(
                        out=OT[r:r + HD, j, lo:hi],
                        in0=po[HD:2 * HD, lo:hi],
                        in1=rbc[0:HD, lo:hi])

                def alloc_u():
                    u_a = upool.tile([128, NT, SPAD], BF16, tag="u", name="ua")
                    u_b = upool.tile([128, NT, SPAD], BF16, tag="u", name="ub")
                    return u_a, u_b

                ns = len(s_tiles)
                emit_kq(0)
                emit_kq(1)
                u_prev = alloc_u()
                for s_i in range(ns):
                    emit_st_si(0, s_i, *u_prev)
                if post_st0 is not None:
                    post_st0()
                for j in range(ND):
                    nxt = j + 1 < ND
                    if j + 2 < ND:
                        emit_kq(j + 2)
                    if nxt:
                        u_cur = alloc_u()
                        emit_st_si(j + 1, 0, *u_cur)
                        emit_st_si(j + 1, 1, *u_cur)
                    po_a = pp.tile([128, D], F32, tag="b", name="po")
                    emit_avmm_half(j, 2 * j, u_prev[0], po_a, 0)
                    if nxt:
                        emit_st_si(j + 1, 2, *u_cur)
                    emit_avmm_half(j, 2 * j, u_prev[0], po_a, 1)
                    emit_norm(j, 2 * j, po_a)
                    if nxt:
                        emit_st_si(j + 1, 3, *u_cur)
                    po_b = pp.tile([128, D], F32, tag="b", name="po")
                    emit_avmm_half(j, 2 * j + 1, u_prev[1], po_b, 0)
                    if nxt:
                        emit_st_si(j + 1, 4, *u_cur)
                    emit_avmm_half(j, 2 * j + 1, u_prev[1], po_b, 1)
                    emit_norm(j, 2 * j + 1, po_b)
                    if nxt:
                        emit_st_si(j + 1, ns - 1, *u_cur)
                        u_prev = u_cur

            # ---------- transformer layers ----------
            # initial LN1 (layer 0); later LN1s fuse into the previous FFN2.
            # V(l) first half is computed in layer l-1's tail (covering the
            # deferred LN1 transposes); the second half inside attention
            # right after S^T(0) (covering pair 0's exp).
            layer_norm_into_hT(ln1g[0] if has_gb else None,
                               ln1b[0] if has_gb else None)
            lin_N(wv[0], hT, ND, evac_v, tiles=[0, 1, 2])

            for l in range(layers):
                last = cls_last and (l == layers - 1)
                ch_q = CH_CLS if last else CH_T

                attention_kq(l, ch_q, post_st0=(
                    lambda l=l: lin_N(wv[l], hT, ND, evac_v, tiles=[3, 4, 5])))

                if not last:
                    pend2 = lin_N(pw[l], OT, ND, evac_accum_x,
                                  chain=make_ln_chain(
                                      ln2g[l] if has_gb else None,
                                      ln2b[l] if has_gb else None,
                                      pb[l] if has_bias else None))
                    if has_bias:
                        fb1_sb = rows.tile([128, NF], F32, tag="fb1")
                        nc.sync.dma_start(out=fb1_sb[:],
                                          in_=fb1[l].rearrange("(t p) -> p t",
                                                               p=128))
                    else:
                        fb1_sb = None
                    # n=0 runs chunk-major: chunk A only needs hT tiles 0-2,
                    # so the pending LN2 transposes slot in after it.
                    lin_T_n(fw1p[l, 0], h3T, 0, relu=True, bias_col=fb1_sb,
                            ch=CH_T3, chunk_pending=pend2)
                    for n in range(1, NF):
                        lin_T_n(fw1p[l, n], h3T, n, relu=True, bias_col=fb1_sb)
                    nl = l + 1
                    pend1 = lin_N(fw2[l], h3T, NF, evac_accum_x,
                                  chain=make_ln_chain(
                                      ln1g[nl] if has_gb else None,
                                      ln1b[nl] if has_gb else None,
                                      fb2[l] if has_bias else None))
                    # V(l+1) first half covers the pending LN1 chains
                    lin_N(wv[nl], hT, ND, evac_v, tiles=[0, 1, 2])
                    for fin in pend1:
                        fin()
                else:
                    # cls-only epilogue: proj, LN2, FFN for the cls token.
                    # DVE can't start at partition 76, so bounce the cls row
                    # through partition 0 with SBUF->SBUF DMAs.
                    def evac_cls(ps):
                        xc = rows.tile([128, D], F32, tag="xc")
                        nc.sync.dma_start(out=xc[0:1, :],
                                          in_=x[CLS_P:CLS_P + 1, CLS_J, :])
                        nc.vector.tensor_add(out=xc[0:1, :], in0=xc[0:1, :],
                                             in1=ps[0:1, :])
                        nc.sync.dma_start(out=x[CLS_P:CLS_P + 1, CLS_J, :],
                                          in_=xc[0:1, :])

                    def lin_cls(w_dram_l, src_sb, k_tiles):
                        """psum[1, 768] = src[:, :, CLS].T @ w ; accum into x."""
                        ps = pp.tile([128, D], F32, tag="b", name="lc")
                        for k in range(k_tiles):
                            wk_t = rhsk.tile([128, D], BF16, tag="rhsk")
                            nc.sync.dma_start(
                                out=wk_t[:],
                                in_=w_dram_l[k * 128:(k + 1) * 128, :])
                            for (c0, w) in CH_D:
                                nc.tensor.matmul(
                                    ps[0:1, c0:c0 + w],
                                    lhsT=src_sb[:, k, CLS:CLS + 1],
                                    rhs=wk_t[:, c0:c0 + w],
                                    start=(k == 0), stop=(k == k_tiles - 1))
                        evac_cls(ps)

                    lin_cls(pw[l], OT, ND)
                    if has_bias:
                        add_bias_to_x(pb[l], tiles=[CLS_J])
                    # LN2 on the cls tile only (row CLS_P of tile CLS_J)
                    layer_norm_into_hT(ln2g[l] if has_gb else None,
                                       ln2b[l] if has_gb else None,
                                       tiles=[CLS_J])
                    if has_bias:
                        fb1_sb = rows.tile([128, NF], F32, tag="fb1")
                        nc.sync.dma_start(out=fb1_sb[:],
                                          in_=fb1[l].rearrange("(t p) -> p t",
                                                               p=128))
                    else:
                        fb1_sb = None
                    for n in range(NF):
                        lin_T_n(fw1p[l, n], h3T, n, relu=True, bias_col=fb1_sb,
                                ch=CH_CLS)
                    lin_cls(fw2[l], h3T, NF)
                    if has_bias:
                        add_bias_to_x(fb2[l], tiles=[CLS_J])

            # ---------- output: cls residual row (row 588 = j4, p76) ----------
            nc.sync.dma_start(out=clsout[:, :], in_=x[CLS_P:CLS_P + 1, CLS_J, :])

    nc.finalize()
    return nc


# ======================= host side =======================

def _sincos_pos(T, d):
    i = np.arange(T, dtype=np.float64)[:, None]
    j = np.arange(d, dtype=np.float64)[None, :]
    je = np.where(j % 2 == 0, j, j - 1)
    ang = i / np.power(10000.0, je / d)
    pe = np.where(j % 2 == 0, np.sin(ang), np.cos(ang))
    return pe.astype(np.float32)


def _patchify_stacked(img):
    b = img.shape[0]
    x = img.reshape(b, IMG // P, P, IMG // P, P, 3, HS)
    x = x.transpose(0, 1, 3, 6, 2, 4, 5)
    return x.reshape(b, NP * HS, P * P * 3)


def _patchify3(img):
    b = img.shape[0]
    x = img.reshape(b, IMG // P, P, IMG // P, P, 3)
    x = x.transpose(0, 1, 3, 2, 4, 5)
    return x.reshape(b, NP, P * P * 3)


def _layernorm_np(v, g, b, eps=1e-5):
    m = v.mean(axis=-1, keepdims=True)
    s = v.var(axis=-1, keepdims=True)
    return (v - m) / np.sqrt(s + eps) * g + b


def _pack_colblocks(w, n_tiles):
    """[L, K, N] -> [L, n_tiles, 128, K] with
    out[l, n, kp, kt*128+c] = w[l, kt*128+kp, n*128+c]."""
    Lb, K, N = w.shape
    kt = K // 128
    v = w.reshape(Lb, kt, 128, n_tiles, 128)
    v = v.transpose(0, 3, 2, 1, 4)          # l, n, kp, kt, c
    return np.ascontiguousarray(v.reshape(Lb, n_tiles, 128, kt * 128))


PERM = np.concatenate([np.arange(2, 394), np.arange(471, 667),
                       np.array([0, 1]), np.arange(394, 471)])


def kernel(**inputs):
    global LAST_EXEC_NS, LAST_TRACE_PATH
    f32 = lambda k: np.asarray(inputs[k], dtype=np.float32)
    bf = lambda a: np.ascontiguousarray(np.asarray(a, dtype=np.float32)
                                        .astype(ml_dtypes.bfloat16))

    has_bias = any(np.any(f32(k)) for k in ("proj_b", "ff_b1", "ff_b2"))
    has_gb = (np.any(f32("ln1_g") != 1.0) or np.any(f32("ln1_b")) or
              np.any(f32("ln2_g") != 1.0) or np.any(f32("ln2_b")))

    key = (has_gb, has_bias, CLS_LAST)
    if key not in _CACHE:
        _CACHE[key] = build_nc(has_gb=has_gb, has_bias=has_bias,
                               cls_last=CLS_LAST)
    nc = _CACHE[key]

    images = f32("images")
    goal_imgs = f32("goal_imgs")
    pose = f32("pose")
    txt = np.asarray(inputs["goals_txt"]).astype(np.int64)
    tok_emb = f32("tok_emb")

    # pose MLP (host, exact fp32 - 4.7 MFLOP)
    pose_tok = np.maximum(pose @ f32("pose_w1") + f32("pose_b1"), 0.0) \
        @ f32("pose_w2") + f32("pose_b2")                       # [B, D]

    pos = _sincos_pos(SEQ, D)                                    # [667, D]
    content = np.zeros((B, SEQ, D), np.float32)
    content[:, 0, :] = f32("cls_tok")[0, 0]
    content[:, 1, :] = pose_tok
    content[:, 2:394, :] = f32("obs_b")
    content[:, 394:471, :] = tok_emb[txt]
    content[:, 471:667, :] = f32("goal_b")
    base = (content + pos[None])[:, PERM, :]                     # permuted
    base_pad = np.zeros((B, TPAD, D), np.float32)
    base_pad[:, :SEQ, :] = base

    p_obs = _patchify_stacked(images)                            # [B, 392, 768]
    p_goal = _patchify3(goal_imgs)                               # [B, 196, 768]
    pobsT = bf(p_obs.transpose(0, 2, 1))                         # [B, 768, 392]
    pgoalT_np = np.zeros((B, D, 204), np.float32)
    pgoalT_np[:, :, 8:] = p_goal.transpose(0, 2, 1)
    pgoalT = bf(pgoalT_np)

    shared = {
        "obs_w": bf(f32("obs_w")), "goal_w": bf(f32("goal_w")),
        "wqp": bf(_pack_colblocks(f32("wq"), ND)),
        "wkp": bf(_pack_colblocks(f32("wk"), ND)),
        "fw1p": bf(_pack_colblocks(f32("ff_w1"), NF)),
        "wv": bf(f32("wv")), "pw": bf(f32("proj_w")), "fw2": bf(f32("ff_w2")),
    }
    if has_bias:
        shared.update({"pb": f32("proj_b"), "fb1": f32("ff_b1"),
                       "fb2": f32("ff_b2")})
    if has_gb:
        shared.update({"ln1g": f32("ln1_g"), "ln1b": f32("ln1_b"),
                       "ln2g": f32("ln2_g"), "ln2b": f32("ln2_b")})
    in_maps = []
    for b in range(B):
        m = dict(shared)
        m["base"] = np.ascontiguousarray(base_pad[b])
        m["pobsT"] = np.ascontiguousarray(pobsT[b])
        m["pgoalT"] = np.ascontiguousarray(pgoalT[b])
        in_maps.append(m)

    res = run_bass_kernel_spmd(nc, in_maps, list(range(B)), trace=TRACE,
                               trace_cores=TRACE_CORES if TRACE else None)
    LAST_EXEC_NS = res.exec_time_ns
    if res.instructions_and_trace is not None:
        LAST_TRACE_PATH = res.instructions_and_trace[1]

    cls = np.stack([np.asarray(res.results[b]["clsout"][0], np.float32)
                    for b in range(B)])                          # [B, D]
    h = _layernorm_np(cls, f32("lnf_g"), f32("lnf_b"))
    h = _layernorm_np(h, f32("hln_g"), f32("hln_b"))
    out = h @ f32("head_w") + f32("head_b")
    return out.astype(np.float32)


# revision 16
# speedup vs baseline: 1.4392x; 1.4392x over previous
"""Trainium2 Bass kernel: ViT-style multimodal transformer (12L, D=768, H=12).

Strategy: pure data parallel - 8 batch elements, one per NeuronCore.
Each core runs the full transformer on its [667, 768] token sequence.

v2 over the previous baseline (2.98ms):
  - All lin_T weight column-blocks are pre-packed on the host so every
    weight DMA is a contiguous [128, 768] row-block on HWDGE (the old
    SWDGE strided loads emitted 768 x 256B descriptors per tile and kept
    the Q7 descriptor engine ~38% busy).
  - LN2 is fused into the proj lin_N and LN1 of layer l+1 into FFN2 of
    layer l (post_tile): the PE transposes interleave with matmul groups
    instead of forming 9us transpose-only windows, which kept re-arming
    the HAM clock throttle (PE was at 1.2GHz 57% of the time).
  - Per-tile LN transposes write one merged [128, 6, 128] bf16 psum tile,
    evacuated with a single ACT copy (6x fewer ACT instructions; ACT has
    a 352-cycle fixed cost per instruction).
  - lin_T evacuations (K/Q copy, FFN1 relu+bias) moved from ACT to DVE.
  - K and Q projections are emitted inside the attention pair pipeline
    (pair j+2's K/Q between pair j's AV halves) so the tensor engine has
    ~8.4us of matmul work per pair to cover the ~10us of exp on ACT.
  - The ones-block in V now occupies columns 0:64, so the softmax
    denominator lands on partitions 0:63 and the DVE reciprocal runs
    directly on the PSUM accumulator (the old ACT partition-bounce is
    gone).

Token order is permuted (attention is permutation-equivariant; positional
embeddings are baked into the additive base): [obs(392) | goal(196) | cls |
pose | text(77)], so patch embeddings land partition-aligned. cls lives at
row 588 = (j=4, p=76).
"""

import numpy as np
import ml_dtypes

import concourse.bass as bass
import concourse.bacc as bacc_mod
import concourse.mybir as mybir
import concourse.tile as tile
from concourse.bass_utils import run_bass_kernel_spmd
from concourse.masks import make_identity

BF16 = mybir.dt.bfloat16
F32 = mybir.dt.float32
AF = mybir.ActivationFunctionType
ALU = mybir.AluOpType

L, H, D, HD = 12, 12, 768, 64
P, IMG, NP, HS = 16, 224, 196, 2
TBLK, VOCAB, POSE_DIM, OUT = 77, 96, 7, 7
B = 8
SEQ = 667          # 1 cls + 1 pose + 392 obs + 77 text + 196 goal
TPAD = 768         # padded token slots (6 partition tiles)
SPAD = 672         # padded free-dim length of transposed activations
NT = 6             # token partition tiles
ND = 6             # feature partition tiles (768/128)
NF = 24            # ffn feature tiles (3072/128)
SCALE = float(D) ** -0.5
EPS = 1e-5
CLS = 588          # permuted cls position = (tile 4, row 76)
CLS_J, CLS_P = 4, 76

# token tiles (start, width)
TT = [(0, 128), (128, 128), (256, 128), (384, 128), (512, 128), (640, 27)]
CH_T = [(0, 512), (512, 155)]   # SEQ chunks (psum bank = 512 fp32)
CH_D = [(0, 512), (512, 256)]   # D chunks
CH_CLS = [(CLS, 1)]             # cls-only chunk (last layer)
# 3-way SEQ split: chunk A only needs token tiles 0-2, B tile 3, C tiles 4-5.
# Offsets keep every psum write inside one 2KB bank.
CH_T3 = [(0, 384), (384, 128), (512, 155)]

# Runtime knobs (test.py may flip these)
TRACE = False
TRACE_CORES = [0]
CLS_LAST = True
LAST_EXEC_NS = None
LAST_TRACE_PATH = None
_CACHE = {}


def _bcast128(ap1d):
    """DMA access pattern broadcasting a 1-D DRAM row across 128 partitions."""
    return bass.AP(tensor=ap1d.tensor, offset=ap1d.offset,
                   ap=[[0, 128]] + list(ap1d.ap))


def build_nc(has_gb=False, has_bias=False, layers=L, cls_last=True):
    nc = bacc_mod.Bacc()

    # ---- per-core data inputs ----
    base = nc.declare_dram_parameter("base", [TPAD, D], F32, isOutput=False)
    pobsT = nc.declare_dram_parameter("pobsT", [D, 392], BF16, isOutput=False)
    pgoalT = nc.declare_dram_parameter("pgoalT", [D, 204], BF16, isOutput=False)
    # ---- shared weights ----
    obs_w = nc.declare_dram_parameter("obs_w", [D, D], BF16, isOutput=False)
    goal_w = nc.declare_dram_parameter("goal_w", [D, D], BF16, isOutput=False)
    # packed column-block layouts: w_p[l, n, kp, kt*128 + c] = w[l, kt*128+kp, n*128+c]
    wqp = nc.declare_dram_parameter("wqp", [L, ND, 128, D], BF16, isOutput=False)
    wkp = nc.declare_dram_parameter("wkp", [L, ND, 128, D], BF16, isOutput=False)
    fw1p = nc.declare_dram_parameter("fw1p", [L, NF, 128, D], BF16, isOutput=False)
    # k-row layouts (lin_N streams row blocks)
    wv = nc.declare_dram_parameter("wv", [L, D, D], BF16, isOutput=False)
    pw = nc.declare_dram_parameter("pw", [L, D, D], BF16, isOutput=False)
    fw2 = nc.declare_dram_parameter("fw2", [L, 4 * D, D], BF16, isOutput=False)
    if has_bias:
        pb = nc.declare_dram_parameter("pb", [L, D], F32, isOutput=False)
        fb1 = nc.declare_dram_parameter("fb1", [L, 4 * D], F32, isOutput=False)
        fb2 = nc.declare_dram_parameter("fb2", [L, D], F32, isOutput=False)
    if has_gb:
        ln1g = nc.declare_dram_parameter("ln1g", [L, D], F32, isOutput=False)
        ln1b = nc.declare_dram_parameter("ln1b", [L, D], F32, isOutput=False)
        ln2g = nc.declare_dram_parameter("ln2g", [L, D], F32, isOutput=False)
        ln2b = nc.declare_dram_parameter("ln2b", [L, D], F32, isOutput=False)
    clsout = nc.declare_dram_parameter("clsout", [1, D], F32, isOutput=True)

    with tile.TileContext(nc) as tc:
        with (
            tc.tile_pool(name="singles", bufs=1) as singles,
            tc.tile_pool(name="wblk", bufs=4) as wblk,    # lin_T weight tiles
            tc.tile_pool(name="rhsk", bufs=12) as rhsk,   # lin_N weight k-tiles
            tc.tile_pool(name="upool", bufs=4) as upool,  # exp(S^T) per head
            tc.tile_pool(name="hn", bufs=6) as hn,        # deferred LN tiles
            tc.tile_pool(name="rows", bufs=2) as rows,
            tc.tile_pool(name="stats", bufs=6) as stats,
            tc.tile_pool(name="lnv", bufs=4) as lnv,
            tc.tile_pool(name="pp", bufs=4, space="PSUM") as pp,
        ):
            # ---------- persistent SBUF ----------
            ident = singles.tile([128, 128], BF16)
            make_identity(nc, ident)
            eps_sb = singles.tile([128, 1], F32)
            nc.vector.memset(eps_sb, EPS)

            x = singles.tile([128, NT, D], F32)            # residual stream
            hT = singles.tile([128, ND, SPAD], BF16)       # LN output, transposed
            QT = singles.tile([128, ND, SPAD], BF16)
            KT = singles.tile([128, ND, SPAD], BF16)
            vbuf = singles.tile([128, NT, H, 2 * HD], BF16)  # ones block + V
            OT = singles.tile([128, ND, SPAD], BF16)       # attn out, transposed
            h3T = singles.tile([128, NF, SPAD], BF16)      # relu ffn hidden, transposed

            # ones block FIRST (cols 0:HD): the AV matmul broadcasts the
            # softmax denominator into PSUM partitions 0..63 so the DVE
            # reciprocal runs there directly (no partition bounce).
            for t_i in range(NT):
                nc.vector.memset(vbuf[:, t_i, :, 0:HD], 1.0)

            # ---------- load residual base ----------
            nc.sync.dma_start(out=x[:], in_=base.rearrange("(j p) d -> p j d", p=128))

            # ---------- patch embeddings ----------
            pobs_sb = singles.tile([128, ND, 392], BF16)
            nc.sync.dma_start(out=pobs_sb[:],
                              in_=pobsT.rearrange("(kt kp) t -> kp kt t", kp=128))
            pgoal_sb = singles.tile([128, ND, 204], BF16)
            nc.sync.dma_start(out=pgoal_sb[:],
                              in_=pgoalT.rearrange("(kt kp) t -> kp kt t", kp=128))

            def embed_add(psrc_sb, w_dram, ptiles, dests):
                # ptiles: list of (col0, width); dests: list of (xrow0, xj)
                for gi in range(0, len(ptiles), 2):
                    grp = list(range(gi, min(gi + 2, len(ptiles))))
                    psums = {}
                    for t_i in grp:
                        psums[t_i] = pp.tile([128, D], F32, tag="b",
                                             name=f"pe{t_i}")
                    for k in range(ND):
                        wk_t = rhsk.tile([128, D], BF16, tag="rhsk")
                        nc.sync.dma_start(out=wk_t[:],
                                          in_=w_dram[k * 128:(k + 1) * 128, :])
                        for t_i in grp:
                            c0, cw = ptiles[t_i]
                            for (s, w) in CH_D:
                                nc.tensor.matmul(
                                    psums[t_i][:cw, s:s + w],
                                    lhsT=psrc_sb[:, k, c0:c0 + cw],
                                    rhs=wk_t[:, s:s + w],
                                    start=(k == 0), stop=(k == ND - 1))
                    for t_i in grp:
                        c0, cw = ptiles[t_i]
                        r0, xj = dests[t_i]
                        nc.vector.tensor_add(
                            out=x[r0:r0 + cw, xj, :],
                            in0=x[r0:r0 + cw, xj, :],
                            in1=psums[t_i][:cw, :])

            embed_add(pobs_sb, obs_w,
                      [(0, 128), (128, 128), (256, 128), (384, 8)],
                      [(0, 0), (0, 1), (0, 2), (0, 3)])
            embed_add(pgoal_sb, goal_w,
                      [(0, 128), (128, 76)],
                      [(0, 3), (0, 4)])

            # ---------- helpers ----------
            def ln_chain_tile(ti, t0, tw, g_bc, b_bc):
                """LN stats + normalize for one tile (DVE/ACT/GPSIMD only).

                Returns a finisher emitting the PE transposes + hT copy;
                the caller defers it under later matmul work so the tensor
                engine never waits on this chain.
                """
                st = stats.tile([128, 3, 6], F32, tag="bnst")
                mv = stats.tile([128, 2], F32, tag="bnmv")
                rstd = stats.tile([128, 1], F32, tag="rstd")
                xi = x[:tw, ti, :].rearrange("p (s c) -> p s c", s=3)
                for s in range(3):
                    nc.vector.bn_stats(out=st[:tw, s, :], in_=xi[:, s, :])
                nc.vector.bn_aggr(out=mv[:tw], in_=st[:tw])
                nc.scalar.activation(out=rstd[:tw], in_=mv[:tw, 1:2],
                                     func=AF.Sqrt, bias=eps_sb[:tw], scale=1.0)
                nc.vector.reciprocal(out=rstd[:tw], in_=rstd[:tw])
                hnat = hn.tile([128, D], BF16, tag="hnat")
                nc.vector.tensor_scalar(out=hnat[:tw], in0=x[:tw, ti, :],
                                        scalar1=mv[:tw, 0:1], scalar2=rstd[:tw],
                                        op0=ALU.subtract, op1=ALU.mult)
                if has_gb:
                    nc.vector.tensor_mul(out=hnat[:tw], in0=hnat[:tw],
                                         in1=g_bc[:tw])
                    nc.vector.tensor_add(out=hnat[:tw], in0=hnat[:tw],
                                         in1=b_bc[:tw])

                def fin():
                    # merged [128, ND, 128] bf16 psum tile, one ACT copy
                    ptile = pp.tile([128, ND, 128], BF16, tag="b", name="pt")
                    for dj in range(ND):
                        nc.tensor.transpose(ptile[:, dj, :tw],
                                            hnat[:tw, dj * 128:(dj + 1) * 128],
                                            ident[:tw, :tw])
                    nc.scalar.activation(out=hT[:, :, t0:t0 + tw],
                                         in_=ptile[:, :, :tw], func=AF.Copy)
                return fin

            def ln_loader(g_dram, b_dram):
                g_bc = b_bc = None
                if has_gb:
                    g_bc = lnv.tile([128, D], F32, tag="g")
                    b_bc = lnv.tile([128, D], F32, tag="bb")
                    nc.sync.dma_start(out=g_bc[:], in_=_bcast128(g_dram))
                    nc.sync.dma_start(out=b_bc[:], in_=_bcast128(b_dram))
                return g_bc, b_bc

            def layer_norm_into_hT(g_dram=None, b_dram=None, tiles=None):
                """Standalone LN (initial layer + cls tile): immediate fin."""
                g_bc, b_bc = ln_loader(g_dram, b_dram)
                for ti, (t0, tw) in enumerate(TT):
                    if tiles is not None and ti not in tiles:
                        continue
                    ln_chain_tile(ti, t0, tw, g_bc, b_bc)()

            def make_ln_chain(g_dram, b_dram, bias_dram=None):
                """chain callback for lin_N: optional residual bias + LN."""
                g_bc, b_bc = ln_loader(g_dram, b_dram)
                bias_bc = None
                if bias_dram is not None:
                    bias_bc = lnv.tile([128, D], F32, tag="xb")
                    nc.sync.dma_start(out=bias_bc[:], in_=_bcast128(bias_dram))

                def chain(t_i, t0, tw):
                    if bias_bc is not None:
                        nc.vector.tensor_add(out=x[:tw, t_i, :],
                                             in0=x[:tw, t_i, :],
                                             in1=bias_bc[:tw])
                    return ln_chain_tile(t_i, t0, tw, g_bc, b_bc)
                return chain

            def lin_T_n(wp_ln, out_sb, n, src=None, relu=False, bias_col=None,
                        ch=CH_T, chunk_pending=None):
                """One n-tile of a transposed-output linear: out[:, n, t].

                wp_ln: packed DRAM slice [128, k_tiles*128] (contiguous).
                chunk_pending: list of finisher closures emitted after the
                first chunk's k-loop (chunk-major mode) - used to slot
                deferred LN transposes under this tile's matmuls.
                """
                if src is None:
                    src = hT
                k_tiles = src.shape[1]
                wb = wblk.tile([128, k_tiles * 128], BF16, tag="wblk")
                nc.sync.dma_start(out=wb[:], in_=wp_ln)
                ps = pp.tile([128, D], F32, tag="b", name="lt")
                if chunk_pending is not None:
                    for ci, (c0, w) in enumerate(ch):
                        for k in range(k_tiles):
                            nc.tensor.matmul(
                                ps[:, c0:c0 + w],
                                lhsT=wb[:, k * 128:(k + 1) * 128],
                                rhs=src[:, k, c0:c0 + w],
                                start=(k == 0), stop=(k == k_tiles - 1))
                        if ci == 0:
                            for fin in chunk_pending:
                                fin()
                else:
                    for k in range(k_tiles):
                        for (c0, w) in ch:
                            nc.tensor.matmul(
                                ps[:, c0:c0 + w],
                                lhsT=wb[:, k * 128:(k + 1) * 128],
                                rhs=src[:, k, c0:c0 + w],
                                start=(k == 0), stop=(k == k_tiles - 1))
                lo, hi = ch[0][0], ch[-1][0] + ch[-1][1]
                if relu:
                    if bias_col is not None:
                        nc.vector.tensor_scalar(
                            out=out_sb[:, n, lo:hi], in0=ps[:, lo:hi],
                            scalar1=bias_col[:, n:n + 1], scalar2=0.0,
                            op0=ALU.add, op1=ALU.max)
                    else:
                        nc.vector.tensor_scalar_max(
                            out=out_sb[:, n, lo:hi], in0=ps[:, lo:hi],
                            scalar1=0.0)
                else:
                    nc.vector.tensor_copy(out=out_sb[:, n, lo:hi],
                                          in_=ps[:, lo:hi])

            def lin_N(w_dram_l, src_sb, k_tiles, evac, tiles=None, chain=None):
                """Natural-layout output: psum[t, 0:768] = src.T @ w per token tile.

                chain(t_i, t0, tw) emits the fused-LN stats chain (DVE/ACT)
                right after each tile's evacuation and returns a finisher
                (PE transposes). Finishers of group g are emitted after
                group g+1's matmuls so the PE never waits on the chain; the
                last group's finishers are RETURNED for the caller to emit
                under the next phase's matmuls.
                """
                tlist = [(t_i, t0, tw) for t_i, (t0, tw) in enumerate(TT)
                         if tiles is None or t_i in tiles]
                prev_fins = []
                for gi in range(0, len(tlist), 3):
                    grp = tlist[gi:gi + 3]
                    psums = {}
                    for (t_i, t0, tw) in grp:
                        psums[t_i] = pp.tile([128, D], F32, tag="b",
                                             name=f"ln{t_i}")
                    for k in range(k_tiles):
                        wk_t = rhsk.tile([128, D], BF16, tag="rhsk")
                        nc.sync.dma_start(out=wk_t[:],
                                          in_=w_dram_l[k * 128:(k + 1) * 128, :])
                        for (t_i, t0, tw) in grp:
                            for (c0, w) in CH_D:
                                nc.tensor.matmul(
                                    psums[t_i][:tw, c0:c0 + w],
                                    lhsT=src_sb[:, k, t0:t0 + tw],
                                    rhs=wk_t[:, c0:c0 + w],
                                    start=(k == 0), stop=(k == k_tiles - 1))
                    fins = []
                    for (t_i, t0, tw) in grp:
                        evac(t_i, t0, tw, psums[t_i])
                        if chain is not None:
                            fins.append(chain(t_i, t0, tw))
                    for fin in prev_fins:
                        fin()
                    prev_fins = fins
                return prev_fins

            def evac_accum_x(t_i, t0, tw, ps):
                nc.vector.tensor_add(out=x[:tw, t_i, :], in0=x[:tw, t_i, :],
                                     in1=ps[:tw, :])

            def evac_v(t_i, t0, tw, ps):
                nc.vector.tensor_copy(
                    out=vbuf[:tw, t_i, :, HD:2 * HD],
                    in_=ps[:tw, :].rearrange("p (h d) -> p h d", d=HD))

            def add_bias_to_x(b_dram_l, tiles=None):
                b_bc = lnv.tile([128, D], F32, tag="xb")
                nc.sync.dma_start(out=b_bc[:], in_=_bcast128(b_dram_l))
                for t_i, (t0, tw) in enumerate(TT):
                    if tiles is not None and t_i not in tiles:
                        continue
                    nc.vector.tensor_add(out=x[:tw, t_i, :], in0=x[:tw, t_i, :],
                                         in1=b_bc[:tw])

            def attention_kq(l, ch, s_tiles=TT, post_st0=None):
                """Attention with K/Q projections interleaved into the pair
                pipeline (writes OT). `ch` is the QUERY chunk list (cls-only
                on the last layer); K is always computed over all tokens.

                Pair j's S^T matmuls run one pair ahead of AV; K/Q of pair
                j+2 are emitted between AV halves so the tensor engine has
                matmul work covering exp's ACT latency. post_st0 (the V
                second half) is emitted right after S^T(0) so pair 0's exp
                runs under ~9us of matmuls. The interleave order also keeps
                the psum pool rotation deadlock-free: every slot's consumer
                is emitted within a few allocations.
                """
                lo, hi = ch[0][0], ch[-1][0] + ch[-1][1]

                def emit_kq(j):
                    lin_T_n(wkp[l, j], KT, j, ch=CH_T)
                    lin_T_n(wqp[l, j], QT, j, ch=ch)

                def emit_st_si(j, s_i, u_a, u_b):
                    s0, sw = s_tiles[s_i]
                    pa = pp.tile([128, D], F32, tag="b", name="pa")
                    pb_ = pp.tile([128, D], F32, tag="b", name="pb")
                    for (c0, w) in ch:
                        nc.tensor.matmul(pa[:sw, c0:c0 + w],
                                         lhsT=KT[0:64, j, s0:s0 + sw],
                                         rhs=QT[0:64, j, c0:c0 + w],
                                         start=True, stop=True)
                        nc.tensor.matmul(pb_[:sw, c0:c0 + w],
                                         lhsT=KT[64:128, j, s0:s0 + sw],
                                         rhs=QT[64:128, j, c0:c0 + w],
                                         start=True, stop=True)
                    nc.scalar.activation(out=u_a[:sw, s_i, lo:hi],
                                         in_=pa[:sw, lo:hi],
                                         func=AF.Exp, scale=SCALE)
                    nc.scalar.activation(out=u_b[:sw, s_i, lo:hi],
                                         in_=pb_[:sw, lo:hi],
                                         func=AF.Exp, scale=SCALE)

                def emit_avmm_half(j, h, u_h, po, half):
                    """Half of the AV matmuls for one head; partitions 0..63
                    get the denominator (ones block of vbuf)."""
                    ns_ = len(s_tiles)
                    rng = range(0, (ns_ + 1) // 2) if half == 0 else \
                        range((ns_ + 1) // 2, ns_)
                    for s_i in rng:
                        s0, sw = s_tiles[s_i]
                        for (c0, w) in ch:
                            nc.tensor.matmul(po[:, c0:c0 + w],
                                             lhsT=vbuf[:sw, s_i, h, :],
                                             rhs=u_h[:sw, s_i, c0:c0 + w],
                                             start=(s_i == 0),
                                             stop=(s_i == ns_ - 1))

                def emit_norm(j, h, po):
                    """OT rows = po[64:128] * 1/po[0:64] (denominator block)."""
                    r = (h % 2) * 64
                    rbc = rows.tile([128, SPAD], F32, tag="rb")
                    nc.vector.reciprocal_approx_fast(
                        out=rbc[0:HD, lo:hi], in_=po[0:HD, lo:hi])
                    nc.vector.tensor_mul(
                        out=OT[r:r + HD, j, lo:hi],
                        in0=po[HD:2 * HD, lo:hi],
                        in1=rbc[0:HD, lo:hi])

                def alloc_u():
                    u_a = upool.tile([128, NT, SPAD], BF16, tag="u", name="ua")
                    u_b = upool.tile([128, NT, SPAD], BF16, tag="u", name="ub")
                    return u_a, u_b

                ns = len(s_tiles)
                emit_kq(0)
                emit_kq(1)
                u_prev = alloc_u()
                for s_i in range(ns):
                    emit_st_si(0, s_i, *u_prev)
                if post_st0 is not None:
                    post_st0()
                for j in range(ND):
                    nxt = j + 1 < ND
                    if j + 2 < ND:
                        emit_kq(j + 2)
                    if nxt:
                        u_cur = alloc_u()
                        emit_st_si(j + 1, 0, *u_cur)
                        emit_st_si(j + 1, 1, *u_cur)
                    po_a = pp.tile([128, D], F32, tag="b", name="po")
                    emit_avmm_half(j, 2 * j, u_prev[0], po_a, 0)
                    if nxt:
                        emit_st_si(j + 1, 2, *u_cur)
                    emit_avmm_half(j, 2 * j, u_prev[0], po_a, 1)
                    emit_norm(j, 2 * j, po_a)
                    if nxt:
                        emit_st_si(j + 1, 3, *u_cur)
                    po_b = pp.tile([128, D], F32, tag="b", name="po")
                    emit_avmm_half(j, 2 * j + 1, u_prev[1], po_b, 0)
                    if nxt:
                        emit_st_si(j + 1, 4, *u_cur)
                    emit_avmm_half(j, 2 * j + 1, u_prev[1], po_b, 1)
                    emit_norm(j, 2 * j + 1, po_b)
                    if nxt:
                        emit_st_si(j + 1, ns - 1, *u_cur)
                        u_prev = u_cur

            # ---------- transformer layers ----------
            # initial LN1 (layer 0); later LN1s fuse into the previous FFN2.
            # V(l) first half is computed in layer l-1's tail (covering the
            # deferred LN1 transposes); the second half inside attention
            # right after S^T(0) (covering pair 0's exp).
            layer_norm_into_hT(ln1g[0] if has_gb else None,
                               ln1b[0] if has_gb else None)
            lin_N(wv[0], hT, ND, evac_v, tiles=[0, 1, 2])

            for l in range(layers):
                last = cls_last and (l == layers - 1)
                ch_q = CH_CLS if last else CH_T

                attention_kq(l, ch_q, post_st0=(
                    lambda l=l: lin_N(wv[l], hT, ND, evac_v, tiles=[3, 4, 5])))

                if not last:
                    pend2 = lin_N(pw[l], OT, ND, evac_accum_x,
                                  chain=make_ln_chain(
                                      ln2g[l] if has_gb else None,
                                      ln2b[l] if has_gb else None,
                                      pb[l] if has_bias else None))
                    if has_bias:
                        fb1_sb = rows.tile([128, NF], F32, tag="fb1")
                        nc.sync.dma_start(out=fb1_sb[:],
                                          in_=fb1[l].rearrange("(t p) -> p t",
                                                               p=128))
                    else:
                        fb1_sb = None
                    # n=0 runs chunk-major: chunk A only needs hT tiles 0-2,
                    # so the pending LN2 transposes slot in after it.
                    lin_T_n(fw1p[l, 0], h3T, 0, relu=True, bias_col=fb1_sb,
                            ch=CH_T3, chunk_pending=pend2)
                    for n in range(1, NF):
                        lin_T_n(fw1p[l, n], h3T, n, relu=True, bias_col=fb1_sb)
                    nl = l + 1
                    pend1 = lin_N(fw2[l], h3T, NF, evac_accum_x,
                                  chain=make_ln_chain(
                                      ln1g[nl] if has_gb else None,
                                      ln1b[nl] if has_gb else None,
                                      fb2[l] if has_bias else None))
                    # V(l+1) first half covers the pending LN1 chains
                    lin_N(wv[nl], hT, ND, evac_v, tiles=[0, 1, 2])
                    for fin in pend1:
                        fin()
                else:
                    # cls-only epilogue: proj, LN2, FFN for the cls token.
                    # DVE can't start at partition 76, so bounce the cls row
                    # through partition 0 with SBUF->SBUF DMAs.
                    def evac_cls(ps):
                        xc = rows.tile([128, D], F32, tag="xc")
                        nc.sync.dma_start(out=xc[0:1, :],
                                          in_=x[CLS_P:CLS_P + 1, CLS_J, :])
                        nc.vector.tensor_add(out=xc[0:1, :], in0=xc[0:1, :],
                                             in1=ps[0:1, :])
                        nc.sync.dma_start(out=x[CLS_P:CLS_P + 1, CLS_J, :],
                                          in_=xc[0:1, :])

                    def lin_cls(w_dram_l, src_sb, k_tiles):
                        """psum[1, 768] = src[:, :, CLS].T @ w ; accum into x."""
                        ps = pp.tile([128, D], F32, tag="b", name="lc")
                        for k in range(k_tiles):
                            wk_t = rhsk.tile([128, D], BF16, tag="rhsk")
                            nc.sync.dma_start(
                                out=wk_t[:],
                                in_=w_dram_l[k * 128:(k + 1) * 128, :])
                            for (c0, w) in CH_D:
                                nc.tensor.matmul(
                                    ps[0:1, c0:c0 + w],
                                    lhsT=src_sb[:, k, CLS:CLS + 1],
                                    rhs=wk_t[:, c0:c0 + w],
                                    start=(k == 0), stop=(k == k_tiles - 1))
                        evac_cls(ps)

                    lin_cls(pw[l], OT, ND)
                    if has_bias:
                        add_bias_to_x(pb[l], tiles=[CLS_J])
                    # LN2 on the cls tile only (row CLS_P of tile CLS_J)
                    layer_norm_into_hT(ln2g[l] if has_gb else None,
                                       ln2b[l] if has_gb else None,
                                       tiles=[CLS_J])
                    if has_bias:
                        fb1_sb = rows.tile([128, NF], F32, tag="fb1")
                        nc.sync.dma_start(out=fb1_sb[:],
                                          in_=fb1[l].rearrange("(t p) -> p t",
                                                               p=128))
                    else:
                        fb1_sb = None
                    for n in range(NF):
                        lin_T_n(fw1p[l, n], h3T, n, relu=True, bias_col=fb1_sb,
                                ch=CH_CLS)
                    lin_cls(fw2[l], h3T, NF)
                    if has_bias:
                        add_bias_to_x(fb2[l], tiles=[CLS_J])

            # ---------- output: cls residual row (row 588 = j4, p76) ----------
            nc.sync.dma_start(out=clsout[:, :], in_=x[CLS_P:CLS_P + 1, CLS_J, :])

    nc.finalize()
    return nc


# ======================= host side =======================

def _sincos_pos(T, d):
    i = np.arange(T, dtype=np.float64)[:, None]
    j = np.arange(d, dtype=np.float64)[None, :]
    je = np.where(j % 2 == 0, j, j - 1)
    ang = i / np.power(10000.0, je / d)
    pe = np.where(j % 2 == 0, np.sin(ang), np.cos(ang))
    return pe.astype(np.float32)


def _patchify_stacked(img):
    b = img.shape[0]
    x = img.reshape(b, IMG // P, P, IMG // P, P, 3, HS)
    x = x.transpose(0, 1, 3, 6, 2, 4, 5)
    return x.reshape(b, NP * HS, P * P * 3)


def _patchify3(img):
    b = img.shape[0]
    x = img.reshape(b, IMG // P, P, IMG // P, P, 3)
    x = x.transpose(0, 1, 3, 2, 4, 5)
    return x.reshape(b, NP, P * P * 3)


def _layernorm_np(v, g, b, eps=1e-5):
    m = v.mean(axis=-1, keepdims=True)
    s = v.var(axis=-1, keepdims=True)
    return (v - m) / np.sqrt(s + eps) * g + b


def _pack_colblocks(w, n_tiles):
    """[L, K, N] -> [L, n_tiles, 128, K] with
    out[l, n, kp, kt*128+c] = w[l, kt*128+kp, n*128+c]."""
    Lb, K, N = w.shape
    kt = K // 128
    v = w.reshape(Lb, kt, 128, n_tiles, 128)
    v = v.transpose(0, 3, 2, 1, 4)          # l, n, kp, kt, c
    return np.ascontiguousarray(v.reshape(Lb, n_tiles, 128, kt * 128))


PERM = np.concatenate([np.arange(2, 394), np.arange(471, 667),
                       np.array([0, 1]), np.arange(394, 471)])


def kernel(**inputs):
    global LAST_EXEC_NS, LAST_TRACE_PATH
    f32 = lambda k: np.asarray(inputs[k], dtype=np.float32)
    bf = lambda a: np.ascontiguousarray(np.asarray(a, dtype=np.float32)
                                        .astype(ml_dtypes.bfloat16))

    has_bias = any(np.any(f32(k)) for k in ("proj_b", "ff_b1", "ff_b2"))
    has_gb = (np.any(f32("ln1_g") != 1.0) or np.any(f32("ln1_b")) or
              np.any(f32("ln2_g") != 1.0) or np.any(f32("ln2_b")))

    key = (has_gb, has_bias, CLS_LAST)
    if key not in _CACHE:
        _CACHE[key] = build_nc(has_gb=has_gb, has_bias=has_bias,
                               cls_last=CLS_LAST)
    nc = _CACHE[key]

    images = f32("images")
    goal_imgs = f32("goal_imgs")
    pose = f32("pose")
    txt = np.asarray(inputs["goals_txt"]).astype(np.int64)
    tok_emb = f32("tok_emb")

    # pose MLP (host, exact fp32 - 4.7 MFLOP)
    pose_tok = np.maximum(pose @ f32("pose_w1") + f32("pose_b1"), 0.0) \
        @ f32("pose_w2") + f32("pose_b2")                       # [B, D]

    pos = _sincos_pos(SEQ, D)                                    # [667, D]
    content = np.zeros((B, SEQ, D), np.float32)
    content[:, 0, :] = f32("cls_tok")[0, 0]
    content[:, 1, :] = pose_tok
    content[:, 2:394, :] = f32("obs_b")
    content[:, 394:471, :] = tok_emb[txt]
    content[:, 471:667, :] = f32("goal_b")
    base = (content + pos[None])[:, PERM, :]                     # permuted
    base_pad = np.zeros((B, TPAD, D), np.float32)
    base_pad[:, :SEQ, :] = base

    p_obs = _patchify_stacked(images)                            # [B, 392, 768]
    p_goal = _patchify3(goal_imgs)                               # [B, 196, 768]
    pobsT = bf(p_obs.transpose(0, 2, 1))                         # [B, 768, 392]
    pgoalT_np = np.zeros((B, D, 204), np.float32)
    pgoalT_np[:, :, 8:] = p_goal.transpose(0, 2, 1)
    pgoalT = bf(pgoalT_np)

    shared = {
        "obs_w": bf(f32("obs_w")), "goal_w": bf(f32("goal_w")),
        "wqp": bf(_pack_colblocks(f32("wq"), ND)),
        "wkp": bf(_pack_colblocks(f32("wk"), ND)),
        "fw1p": bf(_pack_colblocks(f32("ff_w1"), NF)),
        "wv": bf(f32("wv")), "pw": bf(f32("proj_w")), "fw2": bf(f32("ff_w2")),
    }
    if has_bias:
        shared.update({"pb": f32("proj_b"), "fb1": f32("ff_b1"),
                       "fb2": f32("ff_b2")})
    if has_gb:
        shared.update({"ln1g": f32("ln1_g"), "ln1b": f32("ln1_b"),
                       "ln2g": f32("ln2_g"), "ln2b": f32("ln2_b")})
    in_maps = []
    for b in range(B):
        m = dict(shared)
        m["base"] = np.ascontiguousarray(base_pad[b])
        m["pobsT"] = np.ascontiguousarray(pobsT[b])
        m["pgoalT"] = np.ascontiguousarray(pgoalT[b])
        in_maps.append(m)

    res = run_bass_kernel_spmd(nc, in_maps, list(range(B)), trace=TRACE,
                               trace_cores=TRACE_CORES if TRACE else None)
    LAST_EXEC_NS = res.exec_time_ns
    if res.instructions_and_trace is not None:
        LAST_TRACE_PATH = res.instructions_and_trace[1]

    cls = np.stack([np.asarray(res.results[b]["clsout"][0], np.float32)
                    for b in range(B)])                          # [B, D]
    h = _layernorm_np(cls, f32("lnf_g"), f32("lnf_b"))
    h = _layernorm_np(h, f32("hln_g"), f32("hln_b"))
    out = h @ f32("head_w") + f32("head_b")
    return out.astype(np.float32)
